# revision 66
# baseline (speedup 1.0000x reference)
"""LocalLinOSS Trainium2 kernel — 8-core SPMD, sequence-sharded.

Model structure (reference): embedding lookup -> 4 sequential blocks; within a
timestep, block i reads the running hidden h (reset to x_t each step), so the
only cross-time recurrence is the per-block diagonal state
    ns_t = coeff (.) ns_{t-1} + in_to_state @ LN(h_t).
The model therefore decomposes into 4 sequential layer passes over the whole
sequence, each = big matmuls over L (parallel) + a first-order linear scan
(hardware tensor_tensor_scan), followed by the [L,D]@[D,V] output projection.

Sharding: L=4096 split into 8 chunks of T=512 (one per core). Per layer, each
core computes a local scan with zero initial state, AllGathers the 8 chunk
final states (1KB payload), combines them into its carry-in using
host-precomputed decay powers (cdecay^j), and applies the carry as a
correction through the next matmul: s2h @ ns = s2h @ ns_local + (s2h
diag(carry)) @ cpow where cpow[s,t] = c_s^(t+1) — the carry folds into the
s2h weights with one cheap tensor_scalar. The two s2h @ ns_local matmuls are
issued before the collective so they overlap it. The output projection runs per-core on its
own T-chunk in bf16 (fp32 accumulate) and stores fp16 (upcast on host).

Layouts: h lives as [T=4x128 part, D free] (layernorm along free dim);
matmul operands live transposed [D or S part, T free]; PE transposes convert.
bf16 feeds all backbone matmuls (fp32 PSUM accumulate); h, the LN stats and
the scan/carry export path stay float32 (the hw scan keeps fp32 state
internally and only downcasts on write). DMA ring budget: sync ring carries
small consts + layer-0 weights + the carry gathers; layers 1-3 and the
projection weights load on the scalar ring during the first collective's
dead window; the 1KB carry export is copied by the gpsimd engine itself so
the collective trigger follows it on the same queue with no cross-engine
sem hop; the gathered summaries return the same way. The constant ubias
contribution to the scan rides host-precomputed ubcorr tables as extra
carry-independent matmuls (skipped entirely when ubias == 0), and chunk
summaries travel without their (identical) ub part. Run-to-run
variance comes almost entirely from cross-core launch skew absorbed at the
first collective; post-collective time is ~135us.
"""
import os
import sys
sys.path.insert(0, "/opt/trn_rl_repo")
import numpy as np
import concourse.bass as bass
import concourse.bacc as bacc
import concourse.mybir as mybir
import concourse.tile as tile
from concourse.bass_utils import run_bass_kernel_spmd

L, D, S, NB, V = 4096, 256, 256, 4, 8000
NC = 8
T = L // NC            # 512 timesteps per core
P = 128
NT = T // P            # 4 T-tiles per core
NVC = 16               # projection V chunks
VC = V // NVC          # 500
f32 = mybir.dt.float32
f32r = mybir.dt.float32r
bf16 = mybir.dt.bfloat16
f16 = mybir.dt.float16
i32 = mybir.dt.int32
AF = mybir.ActivationFunctionType
OP = mybir.AluOpType

N_WARM1 = 16           # PE keep-warm matmuls during the scan-gate wait
N_WARM2 = 40           # PE keep-warm matmuls during the collective wait

_cache = {}


def _build(use_outb: bool, use_ubc: bool):
    if (nc_cached := _cache.get((use_outb, use_ubc))) is not None:
        return nc_cached
    nc = bacc.Bacc("TRN2", target_bir_lowering=False, debug=False,
                   enable_asserts=False, num_devices=NC)

    def din(name, shape, dtype=f32):
        return nc.dram_tensor(name, shape, dtype, kind="ExternalInput").ap()

    tok_idx = din("tok_idx", [P, NT], i32)
    tok_tab = din("tok_tab", [V, D])
    pos_pre = din("pos_pre", [P, NT * D])
    ident = din("ident", [P, P])
    ident_bf = din("ident_bf", [P, P], bf16)
    ones8 = din("ones8", [8, 1])
    onesT = din("onesT", [1, T], bf16)
    coef_in = din("coef_in", [P, NB * 2])          # coeff, col (i*2+st)
    wmat_in = din("wmat_in", [8, NB * S])          # per-core carry weights
    ubcorr_in = (din("ubcorr_in", [NB, 2, P, T], bf16)  # ub/carry consts
                 if use_ubc else None)
    ddiag_in = din("ddiag_in", [NB, P, 2 * P], bf16)   # diag(direct * ln_w) lhsT
    dbias_in = din("dbias_in", [P, NB * 2])        # direct * ln_b
    opb_in = din("opb_in", [P, NB * 2])            # outp_b
    win_in = din("win_in", [NB, P, 4 * P], bf16)   # lhsT packs
    s2h_in = din("s2h_in", [NB, P, 4 * P], bf16)
    outp_in = din("outp_in", [NB, P, 4 * P], bf16)
    cpow_in = din("cpow_in", [NB, 2, P, T], bf16)  # c^(t+1)
    crev_in = din("crev_in", [NB, 2, P, T], bf16)  # c^(T-1-t)
    outwt_in = din("outwt_in", [2, P, V], bf16)
    outb_in = din("outb_in", [1, V], bf16)
    out_d = nc.dram_tensor("out", [T, V], f16, kind="ExternalOutput").ap()

    with tile.TileContext(nc) as tc:
        with tc.tile_pool(name="const", bufs=1) as cst, \
             tc.tile_pool(name="wts", bufs=1) as wts, \
             tc.tile_pool(name="work", bufs=1) as wk, \
             tc.tile_pool(name="lay", bufs=2) as lay, \
             tc.tile_pool(name="psum", bufs=1, space="PSUM") as ps, \
             tc.tile_pool(name="stage", bufs=1) as stg, \
             tc.tile_pool(name="dram", bufs=1, space="DRAM") as dram:

            # ---- loads, ordered by when they are first needed ----
            ti_sb = wk.tile([P, NT], i32)
            nc.sync.dma_start(ti_sb[:], tok_idx)
            id_f = cst.tile([P, P], f32)
            nc.sync.dma_start(id_f[:], ident)
            id_bf = cst.tile([P, P], bf16)
            nc.sync.dma_start(id_bf[:], ident_bf)
            pos_sb = wk.tile([P, NT * D], f32)
            nc.sync.dma_start(pos_sb[:], pos_pre)
            coef_sb = cst.tile([P, NB * 2], f32)
            nc.sync.dma_start(coef_sb[:], coef_in)
            ones8_sb = cst.tile([8, 1], f32)
            nc.sync.dma_start(ones8_sb[:], ones8)
            wm_sb = cst.tile([8, NB * S], f32)
            nc.sync.dma_start(wm_sb[:], wmat_in)
            db_sb = cst.tile([P, NB * 2], f32)
            nc.sync.dma_start(db_sb[:], dbias_in)
            ob_sb = cst.tile([P, NB * 2], f32)
            nc.sync.dma_start(ob_sb[:], opb_in)
            scrap = cst.tile([1, 1], f32)          # dummy act target
            # prefetch the Identity and Sqrt activation tables while the
            # scalar engine is idle, so layer 0's z/rstd chain never waits;
            # touch the vector queue too so its bring-up cost is paid early
            nc.scalar.activation(scrap[:], id_f[0:1, 0:1], AF.Identity)
            nc.scalar.activation(scrap[:], id_f[0:1, 0:1], AF.Sqrt)
            scrapv = cst.tile([1, 1], f32)
            nc.vector.tensor_copy(scrapv[:], id_f[0:1, 0:1])
            eps_sb = cst.tile([P, 1], f32)
            nc.vector.memset(eps_sb[:], 1e-5)
            # layer-0 weights ride the sync ring now (behind the small
            # consts); layers 1-3 + projection weights are deferred into the
            # first collective's dead window (triggers issued further down on
            # the scalar ring, which sits idle then anyway)
            win_sb, s2h_sb, outp_sb, cpow_sb, ddiag_sb, crev_sb = [], [], [], [], [], []

            def load_layer_weights(i, eng):
                eng.dma_start(win_sb[i][:], win_in[i])
                eng.dma_start(s2h_sb[i][:], s2h_in[i])
                eng.dma_start(outp_sb[i][:], outp_in[i])
                eng.dma_start(ddiag_sb[i][:], ddiag_in[i])
                eng.dma_start(cpow_sb[i][:], cpow_in[i].rearrange("a p t -> p a t"))
                eng.dma_start(crev_sb[i][:], crev_in[i].rearrange("a p t -> p a t"))
                if use_ubc:
                    eng.dma_start(ubc_sb[i][:], ubcorr_in[i].rearrange("a p t -> p a t"))

            ubc_sb = []
            for i in range(NB):
                win_sb.append(wts.tile([P, 4 * P], bf16, name=f"win{i}"))
                s2h_sb.append(wts.tile([P, 4 * P], bf16, name=f"s2h{i}"))
                outp_sb.append(wts.tile([P, 4 * P], bf16, name=f"outp{i}"))
                ddiag_sb.append(wts.tile([P, 2 * P], bf16, name=f"ddiag{i}"))
                cpow_sb.append(wts.tile([P, 2, T], bf16, name=f"cpow{i}"))
                crev_sb.append(wts.tile([P, 2, T], bf16, name=f"crev{i}"))
                if use_ubc:
                    ubc_sb.append(wts.tile([P, 2, T], bf16, name=f"ubc{i}"))
            load_layer_weights(0, nc.sync)
            outwt_sb = [wts.tile([P, V], bf16, name=f"outwt{d}") for d in range(2)]
            if use_outb:
                outb_sb = cst.tile([1, V], bf16)
                ones1_sb = cst.tile([1, P], bf16)
                nc.sync.dma_start(ones1_sb[:], onesT[:, :P])

            # ---- embedding gather + pos add ----
            h = wk.tile([P, NT, D], f32)
            for ct in range(NT):
                nc.gpsimd.indirect_dma_start(
                    out=h[:, ct, :], out_offset=None, in_=tok_tab,
                    in_offset=bass.IndirectOffsetOnAxis(ap=ti_sb[:, ct:ct + 1], axis=0))
                nc.vector.tensor_tensor(
                    h[:, ct, :], h[:, ct, :],
                    pos_sb[:, ct * D:(ct + 1) * D], op=OP.add)

            # warm-up AllGather: absorbs the ~20us first-collective trigger/
            # mesh setup while layer 0 computes, so AG0 itself processes like
            # a steady-state collective. Its desc work occupies the gpsimd
            # queue ~10us starting here — well before the layer-0 export.
            wag_in = dram.tile([8, 1], f32, name="wag_in")
            wag_out = dram.tile([NC, 8, 1], f32, name="wag_out",
                                addr_space="Shared")
            nc.sync.dma_start(wag_in[:], ones8_sb[:])
            nc.gpsimd.collective_compute(
                "AllGather", OP.bypass, replica_groups=[list(range(NC))],
                ins=[wag_in[:]], outs=[wag_out[:]])

            hsT = [None, None]

            # ---- 4 sequential layer passes ----
            for i in range(NB):
                last = i == NB - 1
                # 1. layernorm stats + z = (h - mean) * rstd   (z in bf16)
                z = lay.tile([P, NT, D], bf16, tag="z")
                stats = lay.tile([P, NT, 6], f32, tag="stats")
                aggr = lay.tile([P, NT, 2], f32, tag="aggr")
                rstd = lay.tile([P, NT], f32, tag="rstd")
                negmr = lay.tile([P, NT], f32, tag="negmr")
                # rstd/z run in two ct-halves: the first half's z writes
                # overlap the second half's bn_stats, and each sqrt hop hides
                # behind the other half's vector work
                for hh in range(2):
                    cs = slice(hh * 2, hh * 2 + 2)
                    for ct in range(hh * 2, hh * 2 + 2):
                        nc.vector.bn_stats(stats[:, ct, :], h[:, ct, :])
                        nc.vector.bn_aggr(aggr[:, ct, :], stats[:, ct, :])
                    nc.scalar.activation(rstd[:, cs], aggr[:, cs, 1], AF.Sqrt,
                                         bias=eps_sb[:, 0:1])
                    nc.vector.reciprocal(rstd[:, cs], rstd[:, cs])
                    # negmr = -mean * rstd  (z = h*rstd + negmr)
                    nc.vector.scalar_tensor_tensor(
                        negmr[:, cs], aggr[:, cs, 0], -1.0, rstd[:, cs],
                        op0=OP.mult, op1=OP.mult)
                    for ct in range(hh * 2, hh * 2 + 2):
                        if ct % 2 == 0:
                            nc.scalar.activation(z[:, ct, :], h[:, ct, :],
                                                 AF.Identity,
                                                 bias=negmr[:, ct:ct + 1],
                                                 scale=rstd[:, ct:ct + 1])
                        else:
                            nc.vector.tensor_scalar(z[:, ct, :], h[:, ct, :],
                                                    rstd[:, ct:ct + 1],
                                                    negmr[:, ct:ct + 1],
                                                    op0=OP.mult, op1=OP.add)
                # 2. transpose z -> zT [D part, T free]; ct-outer issue order
                # so the PE FIFO never parks ready transposes behind a z tile
                # that is still being written
                zT = [lay.tile([P, T], bf16, tag=f"zT{d}", name=f"zT{d}") for d in range(2)]
                zt_ps = [ps.tile([P, T], bf16, tag="pp", bufs=8, name=f"zt_ps{d}")
                         for d in range(2)]
                for ct in range(NT):
                    for d in range(2):
                        nc.tensor.transpose(zt_ps[d][:, ct * P:(ct + 1) * P],
                                            z[:, ct, d * P:(d + 1) * P], id_bf[:])
                nc.scalar.activation(zT[0][:], zt_ps[0][:], AF.Identity)
                nc.vector.tensor_copy(zT[1][:], zt_ps[1][:])
                # 3. u = W_in' @ z (+ubias via ones-row matmul); the chunk
                # summary (last scan state) = sum_k c^(T-1-k) u_k computed by a
                # fused multiply+accumulate so the export never waits on a scan
                ns1 = [lay.tile([P, T], bf16, tag=f"ns1{st}", name=f"ns1{st}")
                       for st in range(2)]
                last2 = lay.tile([P, 2], f32, tag="last2")
                scrT = lay.tile([P, T], bf16, tag="scrT")
                u_ps = [None, None]
                for st in range(2):
                    u_ps[st] = ps.tile([P, T], f32, tag="pp", bufs=8, name=f"u_ps{st}")
                    for kt in range(2):
                        nc.tensor.matmul(u_ps[st][:],
                                         win_sb[i][:, (kt * 2 + st) * P:(kt * 2 + st + 1) * P],
                                         zT[kt][:], start=(kt == 0), stop=(kt == 1))
                for st in range(2):
                    nc.vector.scalar_tensor_tensor(
                        scrT[:], u_ps[st][:], 1.0, crev_sb[i][:, st, :],
                        op0=OP.bypass, op1=OP.mult,
                        accum_out=last2[:, st:st + 1])
                # 4. export chunk-final states (transposed: 2 descriptors),
                #    AllGather, combine into carry
                lt_ps = ps.tile([2, P], f32, tag="pp", bufs=8, name="lt_ps")
                nc.tensor.transpose(lt_ps[:], last2[:], id_f[:])
                exp_sb = lay.tile([2, P], bf16, tag="exp")
                nc.scalar.activation(exp_sb[:], lt_ps[:], AF.Identity)
                ag_in = dram.tile([2, P], bf16, name=f"ag_in{i}")
                ag_out = dram.tile([NC, 2, P], bf16, name=f"ag_out{i}",
                                   addr_space="Shared")
                # gpsimd copies the 1KB export itself: the collective trigger
                # is next on the same queue, skipping a cross-engine sem hop
                nc.gpsimd.dma_start(ag_in[:], exp_sb[:])
                if i == 0:
                    # bulk weight loads ride out the collective dead window;
                    # the scalar engine has nothing to do until the carry lands
                    for j in range(1, NB):
                        load_layer_weights(j, nc.scalar)
                    for d in range(2):
                        nc.scalar.dma_start(outwt_sb[d][:], outwt_in[d])
                    if use_outb:
                        nc.scalar.dma_start(outb_sb[:], outb_in)
                # gelu table prefetch sits in the collective dead window; the
                # dep on exp_sb pins it there (a dep on z gets hoisted into
                # the z -> zT -> u critical chain by the scheduler)
                nc.scalar.activation(scrap[:], exp_sb[0:1, 0:1], AF.Gelu_apprx_tanh)
                # full local scans: only the mixed matmuls need them, so they
                # run during the collective wait. The zero initial value is
                # computed from last2 purely to gate the scans behind both
                # summary accumulations — a greedy scheduler otherwise slots a
                # 1.2us scan between them and delays the export.
                z0 = lay.tile([P, 1], f32, tag="z0")
                nc.vector.scalar_tensor_tensor(z0[:], last2[:, 0:1], 0.0,
                                               last2[:, 1:2],
                                               op0=OP.mult, op1=OP.mult)
                for st in range(2):
                    cb = coef_sb[:, i * 2 + st:i * 2 + st + 1].to_broadcast((P, T))
                    nc.vector.tensor_tensor_scan(ns1[st][:], cb, u_ps[st][:],
                                                 z0[:, 0:1],
                                                 op0=OP.mult, op1=OP.add)
                # PE keep-warm: fine-grained dummy matmuls hold the HAM clock
                # gate at 8/8 through the two PE stalls (scan-gate wait, then
                # the collective wait) with <=220ns of FIFO drain each
                # full-bank tile: a fractional PSUM tile can land in a bank
                # with an open m_ps accumulation group and corrupt it
                warm_ps = ps.tile([P, T], f32, tag="pp", bufs=8, name="warm_ps")
                for w in range(N_WARM1):
                    nc.tensor.matmul(warm_ps[:, :P], id_bf[:], zT[0][:, :P],
                                     start=True, stop=True)
                # mixed-psum partials that don't need the carry: issued now so
                # they run during the collective (PE queue is FIFO)
                mixed = [lay.tile([P, T], bf16, tag=f"mix{d}", name=f"mix{d}")
                         for d in range(2)]
                m_ps = [None, None]
                for d in range(2):
                    m_ps[d] = ps.tile([P, T], f32, tag="pp", bufs=8, name=f"m_ps{d}")
                    for st in range(2):
                        nc.tensor.matmul(m_ps[d][:],
                                         s2h_sb[i][:, (st * 2 + d) * P:(st * 2 + d + 1) * P],
                                         ns1[st][:], start=(st == 0), stop=False)
                    nc.tensor.matmul(m_ps[d][:],
                                     ddiag_sb[i][:, d * P:(d + 1) * P],
                                     zT[d][:], start=False, stop=False)
                    if use_ubc:
                        # constant ubias prefix: carry-independent, so it
                        # rides the collective dead window
                        for st in range(2):
                            nc.tensor.matmul(
                                m_ps[d][:],
                                s2h_sb[i][:, (st * 2 + d) * P:(st * 2 + d + 1) * P],
                                ubc_sb[i][:, st, :], start=False, stop=False)
                if last:
                    # h^T transposes are carry-independent: run them in the
                    # collective dead window, not behind the carry matmuls
                    hT_ps_l = []
                    for d2 in range(2):
                        hT = ps.tile([P, T], f32, tag="pp", bufs=8,
                                     name=f"hT_ps{d2}")
                        for ct in range(NT):
                            nc.tensor.transpose(hT[:, ct * P:(ct + 1) * P],
                                                h[:, ct, d2 * P:(d2 + 1) * P],
                                                id_f[:])
                        hT_ps_l.append(hT)
                for w in range(150 if i == 0 else N_WARM2):
                    nc.tensor.matmul(warm_ps[:, :P], id_bf[:], zT[0][:, :P],
                                     start=True, stop=True)
                nc.gpsimd.collective_compute(
                    "AllGather", OP.bypass, replica_groups=[list(range(NC))],
                    ins=[ag_in[:]], outs=[ag_out[:]])
                # gpsimd also copies the gathered summaries back: the copy
                # follows the collective on the same queue, no sem hop
                gath = lay.tile([8, S], bf16, tag="gath")
                nc.gpsimd.dma_start(gath[:], ag_out[:].rearrange("c a b -> c (a b)"))
                q = lay.tile([8, S], f32, tag="q")
                nc.vector.tensor_tensor(q[:], wm_sb[:, i * S:(i + 1) * S], gath[:],
                                        op=OP.mult)
                c_ps = [None, None]
                s2hc = lay.tile([P, 4 * P], bf16, tag="s2hc", name="s2hc")
                for st in range(2):
                    c_ps[st] = ps.tile([P, 1], f32, tag="pp", bufs=8, name=f"c_ps{st}")
                    nc.tensor.matmul(c_ps[st][:], q[:, st * P:(st + 1) * P],
                                     ones8_sb[:], start=True, stop=True)
                    # fold the carry into the s2h weights (s2h diag(carry) @
                    # c^(t+1) == s2h @ (c^(t+1) (.) carry)): one cheap [P,2P]
                    # tensor_scalar instead of building full [P,T] A tables
                    nc.vector.tensor_scalar_mul(
                        s2hc[:, st * 2 * P:(st + 1) * 2 * P],
                        s2h_sb[i][:, st * 2 * P:(st + 1) * 2 * P],
                        c_ps[st][:, 0:1])
                # 6. finish mixed: carry matmuls against the c^(t+1) table,
                # then gelu straight off PSUM
                for d in range(2):
                    for st in range(2):
                        nc.tensor.matmul(m_ps[d][:],
                                         s2hc[:, (st * 2 + d) * P:(st * 2 + d + 1) * P],
                                         cpow_sb[i][:, st, :], start=False, stop=(st == 1))
                    nc.scalar.activation(mixed[d][:], m_ps[d][:], AF.Gelu_apprx_tanh,
                                         bias=db_sb[:, i * 2 + d:i * 2 + d + 1])
                # 6. delta = outp_W' @ mixed (+outp_b); last layer's delta is
                # consumed by a plain f32 add, earlier layers by a PE transpose
                delta = [lay.tile([P, T], f32 if last else bf16,
                                  tag=f"del{d}", name=f"del{d}")
                         for d in range(2)]
                for d2 in range(2):
                    d_ps = ps.tile([P, T], f32, tag="pp", bufs=8, name=f"d_ps{d2}")
                    for d in range(2):
                        nc.tensor.matmul(d_ps[:],
                                         outp_sb[i][:, (d * 2 + d2) * P:(d * 2 + d2 + 1) * P],
                                         mixed[d][:], start=(d == 0), stop=(d == 1))
                    for hh in range(2):
                        sl = slice(hh * (T // 2), (hh + 1) * (T // 2))
                        if d2 == 0:
                            nc.scalar.activation(delta[d2][:, sl], d_ps[:, sl],
                                                 AF.Identity,
                                                 bias=ob_sb[:, i * 2 + d2:i * 2 + d2 + 1])
                        else:
                            nc.vector.tensor_scalar(delta[d2][:, sl], d_ps[:, sl],
                                                    ob_sb[:, i * 2 + d2:i * 2 + d2 + 1],
                                                    None, op0=OP.add)
                if not last:
                    nc.scalar.activation(scrap[:], delta[1][0:1, 0:1], AF.Sqrt)
                # 7. residual
                if not last:
                    for ct in range(NT):
                        dT_ps = ps.tile([P, D], bf16, tag="pp", bufs=8, name="dT_ps")
                        for d2 in range(2):
                            nc.tensor.transpose(dT_ps[:, d2 * P:(d2 + 1) * P],
                                                delta[d2][:, ct * P:(ct + 1) * P], id_bf[:])
                        (nc.vector if ct % 2 == 0 else nc.vector).tensor_tensor(
                            h[:, ct, :], h[:, ct, :], dT_ps[:], op=OP.add)
                else:
                    # hsT = h^T + delta in [D part, T free], bf16 for projection
                    for d2 in range(2):
                        hsT[d2] = wk.tile([P, T], bf16, name=f"hsT{d2}")
                        # per-mt adds: the projection's first lhsT block only
                        # waits for its own 128 columns
                        for ct in range(NT):
                            nc.vector.tensor_tensor(
                                hsT[d2][:, ct * P:(ct + 1) * P],
                                delta[d2][:, ct * P:(ct + 1) * P],
                                hT_ps_l[d2][:, ct * P:(ct + 1) * P], op=OP.add)

            # ---- output projection: out[t, v] = hsT[:, t] . outwt[:, v] ----
            for mt in range(NT):
                for vg in range(NVC // 4):
                    st_t = stg.tile([P, 4 * VC], f16, tag="stg", bufs=4)
                    for vs in range(4):
                        vc = vg * 4 + vs
                        p_ps = ps.tile([P, VC], f32, tag="pp", bufs=8, name="p_ps")
                        for d in range(2):
                            nc.tensor.matmul(p_ps[:], hsT[d][:, mt * P:(mt + 1) * P],
                                             outwt_sb[d][:, vc * VC:(vc + 1) * VC],
                                             start=(d == 0),
                                             stop=(d == 1 and not use_outb))
                        if use_outb:
                            nc.tensor.matmul(p_ps[:], ones1_sb[:],
                                             outb_sb[:, vc * VC:(vc + 1) * VC],
                                             start=False, stop=True)
                        if vc % 2 == 0:
                            nc.vector.tensor_copy(st_t[:, vs * VC:(vs + 1) * VC], p_ps[:])
                        else:
                            nc.scalar.activation(st_t[:, vs * VC:(vs + 1) * VC], p_ps[:],
                                                 AF.Identity)
                    if mt == NT - 1 and vg == NVC // 4 - 1:
                        # final group: two half stores on both rings in
                        # parallel to shorten the drain tail
                        nc.sync.dma_start(
                            out_d[mt * P:(mt + 1) * P,
                                  vg * 4 * VC:vg * 4 * VC + 2 * VC],
                            st_t[:, :2 * VC])
                        nc.scalar.dma_start(
                            out_d[mt * P:(mt + 1) * P,
                                  vg * 4 * VC + 2 * VC:(vg + 1) * 4 * VC],
                            st_t[:, 2 * VC:])
                    else:
                        (nc.sync if vg % 2 == 0 else nc.scalar).dma_start(
                            out_d[mt * P:(mt + 1) * P,
                                  vg * 4 * VC:(vg + 1) * 4 * VC],
                            st_t[:])

    nc.compile()
    _cache[(use_outb, use_ubc)] = nc
    return nc


def _pack_lhsT(w):
    """w: [M, K] weight for out = w @ x. Returns [128, (K/128)*(M/128)*128] lhsT pack;
    block b = kt*nmt + mt holds lhsT[kt*128+p, mt*128+m]."""
    M, K = w.shape
    lhsT = np.ascontiguousarray(w.T)                       # [K, M]
    t = lhsT.reshape(K // P, P, M // P, P)                 # [kt, p, mt, m]
    return np.ascontiguousarray(t.transpose(1, 0, 2, 3).reshape(P, -1))


def kernel(**inputs):
    xs = {k: np.asarray(v) for k, v in inputs.items()}
    tokens = xs["tokens"].astype(np.int32)
    token_embed = xs["token_embed"].astype(np.float32)
    pos_embed = xs["pos_embed"].astype(np.float32)
    in_to_state = xs["in_to_state"].astype(np.float64)
    state_to_hidden = xs["state_to_hidden"].astype(np.float64)
    direct = xs["direct"].astype(np.float64)
    a_diag = xs["a_diag"].astype(np.float64)
    g_diag = xs["g_diag"].astype(np.float64)
    dtp = xs["dt"].astype(np.float64)
    ln_w = xs["ln_w"].astype(np.float64)
    ln_b = xs["ln_b"].astype(np.float64)
    outp_W = xs["outp_W"].astype(np.float64)
    outp_b = xs["outp_b"].astype(np.float32)
    out_W = xs["out_W"].astype(np.float32)
    out_b = xs["out_b"].astype(np.float32)

    def softplus(x):
        return np.logaddexp(0.0, x)

    dt_e = softplus(dtp) + 1e-4
    coeff = np.exp(-softplus(g_diag) * dt_e) * np.cos(a_diag * dt_e)   # [NB, S]
    cdecay = coeff ** T                                                 # [NB, S]
    # c^(t+1) tables for the carry correction, [NB, 2, P, T]
    tpow = np.arange(1, T + 1, dtype=np.float64)
    cpow = coeff.reshape(NB, 2, P, 1) ** tpow.reshape(1, 1, 1, T)
    trev = np.arange(T - 1, -1, -1, dtype=np.float64)
    crev = coeff.reshape(NB, 2, P, 1) ** trev.reshape(1, 1, 1, T)

    import ml_dtypes
    bfl = ml_dtypes.bfloat16
    # packed weights (shared across cores)
    win_pack = np.stack([_pack_lhsT(in_to_state[i] * ln_w[i][None, :]) for i in range(NB)]).astype(bfl)
    s2h_pack = np.stack([_pack_lhsT(state_to_hidden[i]) for i in range(NB)]).astype(bfl)
    outp_pack = np.stack([_pack_lhsT(outp_W[i]) for i in range(NB)]).astype(bfl)
    outwt_pack = np.ascontiguousarray(out_W.T.reshape(2, P, V))
    outwt_bf16 = outwt_pack.astype(bfl)
    ubias = np.stack([in_to_state[i] @ ln_b[i] for i in range(NB)])     # [NB, S]
    # geometric prefixes of coeff: p_t = c p_(t-1) + 1, so the constant ubias
    # feeds the scan as ns_true = scan(u_raw) + ubias * p  (folded into the
    # A-table add) and the chunk summary as + ubias * p_(T-1)
    pref = np.ones((NB, S, T), np.float64)
    for t in range(1, T):
        pref[:, :, t] = coeff * pref[:, :, t - 1] + 1.0
    ubpre = ubias[:, :, None] * pref               # [NB, S, T] local ub prefix
    lbcflat = ubias * pref[:, :, T - 1]            # ub part of every summary
    dprime = direct * ln_w                                              # [NB, D]
    dbias = direct * ln_b                                               # [NB, D]

    def cols(v):  # [NB, 256] -> [128, NB*2] with col (i*2+half)
        return np.ascontiguousarray(
            v.reshape(NB, 2, P).transpose(2, 0, 1).reshape(P, NB * 2)).astype(np.float32)

    use_outb = bool(np.any(out_b != 0.0))

    base = dict(
        tok_tab=token_embed, ident=np.eye(P, dtype=np.float32),
        ident_bf=np.eye(P, dtype=np.float32).astype(bfl),
        ones8=np.ones((8, 1), np.float32),
        onesT=np.ones((1, T), bfl),
        coef_in=cols(coeff),
        ddiag_in=np.ascontiguousarray(np.concatenate(
            [np.stack([np.diag(dprime[i, d * P:(d + 1) * P]) for d in range(2)],
                      axis=1).reshape(P, 2 * P)[None] for i in range(NB)])
        ).astype(bfl),
        dbias_in=cols(dbias),
        opb_in=cols(np.broadcast_to(outp_b, (NB, D)).astype(np.float64)),
        win_in=win_pack, s2h_in=s2h_pack, outp_in=outp_pack,
        cpow_in=cpow.astype(bfl),
        crev_in=crev.astype(bfl),
        outwt_in=outwt_bf16, outb_in=out_b.reshape(1, V).astype(bfl),
    )

    in_maps = []
    for k in range(NC):
        sl = slice(k * T, (k + 1) * T)
        tk = tokens[sl].reshape(NT, P).T.copy()            # [128, NT]
        pos = np.ascontiguousarray(
            pos_embed[sl].reshape(NT, P, D).transpose(1, 0, 2).reshape(P, NT * D))
        # carry weights: wmat[j, s] = cdecay[s]^(k-1-j) for j<k else 0
        wm = np.zeros((8, NB, S), np.float64)
        for j in range(k):
            wm[j] = cdecay ** (k - 1 - j)
        # summaries travel without their (identical) ub part; receivers fold
        # lbc * sum_j(wm) into the carry correction table instead
        corr = lbcflat * wm.sum(0)                 # [NB, S]
        m = dict(base, tok_idx=tk, pos_pre=pos,
                 wmat_in=wm.reshape(8, NB * S).astype(np.float32))
        if np.any(ubias != 0.0):
            ubc_k = ubpre + corr[:, :, None] * cpow.reshape(NB, S, T)
            m["ubcorr_in"] = ubc_k.reshape(NB, 2, P, T).astype(bfl)
        in_maps.append(m)

    use_ubc = bool(np.any(ubias != 0.0))
    nc = _build(use_outb, use_ubc)
    trace = bool(os.environ.get("BASS_KERNEL_TRACE"))
    res = run_bass_kernel_spmd(nc, in_maps, core_ids=list(range(NC)), trace=trace)
    if trace:
        kernel.last_exec_time_ns = res.exec_time_ns
        kernel.last_results = res
    return np.concatenate(
        [res.results[k]["out"].astype(np.float32) for k in range(NC)], axis=0)


# revision 67
# speedup vs baseline: 1.3115x; 1.3115x over previous
"""LocalLinOSS Trainium2 kernel — 8-core SPMD, sequence-sharded.

Model structure (reference): embedding lookup -> 4 sequential blocks; within a
timestep, block i reads the running hidden h (reset to x_t each step), so the
only cross-time recurrence is the per-block diagonal state
    ns_t = coeff (.) ns_{t-1} + in_to_state @ LN(h_t).
The model therefore decomposes into 4 sequential layer passes over the whole
sequence, each = big matmuls over L (parallel) + a first-order linear scan
(hardware tensor_tensor_scan), followed by the [L,D]@[D,V] output projection.

Sharding: L=4096 split into 8 chunks of T=512 (one per core). Per layer, each
core computes a local scan with zero initial state, AllGathers the 8 chunk
final states (1KB payload), combines them into its carry-in using
host-precomputed decay powers (cdecay^j), and applies the carry as a
correction through the next matmul: s2h @ ns = s2h @ ns_local + (s2h
diag(carry)) @ cpow where cpow[s,t] = c_s^(t+1) — the carry folds into the
s2h weights with one cheap tensor_scalar. The two s2h @ ns_local matmuls are
issued before the collective so they overlap it. The output projection runs per-core on its
own T-chunk in bf16 (fp32 accumulate) and stores fp16 (upcast on host).

Layouts: h lives as [T=4x128 part, D free] (layernorm along free dim);
matmul operands live transposed [D or S part, T free]; PE transposes convert.
bf16 feeds all backbone matmuls (fp32 PSUM accumulate); h, the LN stats and
the scan/carry export path stay float32 (the hw scan keeps fp32 state
internally and only downcasts on write). DMA ring budget: sync ring carries
small consts + layer-0 weights + the carry gathers; layers 1-3 and the
projection weights load on the scalar ring during the first collective's
dead window; the 1KB carry export is copied by the gpsimd engine itself so
the collective trigger follows it on the same queue with no cross-engine
sem hop; the gathered summaries return the same way. The constant ubias
contribution to the scan rides host-precomputed ubcorr tables as extra
carry-independent matmuls (skipped entirely when ubias == 0), and chunk
summaries travel without their (identical) ub part. Run-to-run
variance comes almost entirely from cross-core launch skew absorbed at the
first collective; post-collective time is ~135us.
"""
import os
import sys
sys.path.insert(0, "/opt/trn_rl_repo")
import numpy as np
import concourse.bass as bass
import concourse.bacc as bacc
import concourse.mybir as mybir
import concourse.tile as tile
from concourse.bass_utils import run_bass_kernel_spmd

L, D, S, NB, V = 4096, 256, 256, 4, 8000
NC = 8
T = L // NC            # 512 timesteps per core
P = 128
NT = T // P            # 4 T-tiles per core
NVC = 16               # projection V chunks
VC = V // NVC          # 500
f32 = mybir.dt.float32
f32r = mybir.dt.float32r
bf16 = mybir.dt.bfloat16
f16 = mybir.dt.float16
i32 = mybir.dt.int32
AF = mybir.ActivationFunctionType
OP = mybir.AluOpType

N_WARM1 = 16           # PE keep-warm matmuls during the scan-gate wait
N_WARM2 = 40           # PE keep-warm matmuls during the collective wait

_cache = {}


def _build(use_outb: bool, use_ubc: bool):
    if (nc_cached := _cache.get((use_outb, use_ubc))) is not None:
        return nc_cached
    nc = bacc.Bacc("TRN2", target_bir_lowering=False, debug=False,
                   enable_asserts=False, num_devices=NC)

    def din(name, shape, dtype=f32):
        return nc.dram_tensor(name, shape, dtype, kind="ExternalInput").ap()

    tok_idx = din("tok_idx", [P, NT], i32)
    tok_tab = din("tok_tab", [V, D])
    pos_pre = din("pos_pre", [P, NT * D])
    ident = din("ident", [P, P])
    ident_bf = din("ident_bf", [P, P], bf16)
    ones8 = din("ones8", [8, 1])
    onesT = din("onesT", [1, T], bf16)
    coef_in = din("coef_in", [P, NB * 2])          # coeff, col (i*2+st)
    wmat_in = din("wmat_in", [8, NB * S])          # per-core carry weights
    ubcorr_in = (din("ubcorr_in", [NB, 2, P, T], bf16)  # ub/carry consts
                 if use_ubc else None)
    ddiag_in = din("ddiag_in", [NB, P, 2 * P], bf16)   # diag(direct * ln_w) lhsT
    dbias_in = din("dbias_in", [P, NB * 2])        # direct * ln_b
    opb_in = din("opb_in", [P, NB * 2])            # outp_b
    win_in = din("win_in", [NB, P, 4 * P], bf16)   # lhsT packs
    s2h_in = din("s2h_in", [NB, P, 4 * P], bf16)
    outp_in = din("outp_in", [NB, P, 4 * P], bf16)
    cpow_in = din("cpow_in", [NB, 2, P, T], bf16)  # c^(t+1)
    crev_in = din("crev_in", [NB, 2, P, T], bf16)  # c^(T-1-t)
    outwt_in = din("outwt_in", [2, P, V], bf16)
    outb_in = din("outb_in", [1, V], bf16)
    out_d = nc.dram_tensor("out", [T, V], f16, kind="ExternalOutput").ap()

    with tile.TileContext(nc) as tc:
        with tc.tile_pool(name="const", bufs=1) as cst, \
             tc.tile_pool(name="wts", bufs=1) as wts, \
             tc.tile_pool(name="work", bufs=1) as wk, \
             tc.tile_pool(name="lay", bufs=2) as lay, \
             tc.tile_pool(name="psum", bufs=1, space="PSUM") as ps, \
             tc.tile_pool(name="stage", bufs=1) as stg, \
             tc.tile_pool(name="dram", bufs=1, space="DRAM") as dram:

            # ---- loads, ordered by when they are first needed ----
            ti_sb = wk.tile([P, NT], i32)
            nc.sync.dma_start(ti_sb[:], tok_idx)
            id_f = cst.tile([P, P], f32)
            nc.sync.dma_start(id_f[:], ident)
            id_bf = cst.tile([P, P], bf16)
            nc.sync.dma_start(id_bf[:], ident_bf)
            pos_sb = wk.tile([P, NT * D], f32)
            nc.sync.dma_start(pos_sb[:], pos_pre)
            coef_sb = cst.tile([P, NB * 2], f32)
            nc.sync.dma_start(coef_sb[:], coef_in)
            ones8_sb = cst.tile([8, 1], f32)
            nc.sync.dma_start(ones8_sb[:], ones8)
            wm_sb = cst.tile([8, NB * S], f32)
            nc.sync.dma_start(wm_sb[:], wmat_in)
            db_sb = cst.tile([P, NB * 2], f32)
            nc.sync.dma_start(db_sb[:], dbias_in)
            ob_sb = cst.tile([P, NB * 2], f32)
            nc.sync.dma_start(ob_sb[:], opb_in)
            scrap = cst.tile([1, 1], f32)          # dummy act target
            # prefetch the Identity and Sqrt activation tables while the
            # scalar engine is idle, so layer 0's z/rstd chain never waits;
            # touch the vector queue too so its bring-up cost is paid early
            nc.scalar.activation(scrap[:], id_f[0:1, 0:1], AF.Identity)
            nc.scalar.activation(scrap[:], id_f[0:1, 0:1], AF.Sqrt)
            scrapv = cst.tile([1, 1], f32)
            nc.vector.tensor_copy(scrapv[:], id_f[0:1, 0:1])
            eps_sb = cst.tile([P, 1], f32)
            nc.vector.memset(eps_sb[:], 1e-5)
            # layer-0 weights ride the sync ring now (behind the small
            # consts); layers 1-3 + projection weights are deferred into the
            # first collective's dead window (triggers issued further down on
            # the scalar ring, which sits idle then anyway)
            win_sb, s2h_sb, outp_sb, cpow_sb, ddiag_sb, crev_sb = [], [], [], [], [], []

            def load_layer_weights(i, eng):
                eng.dma_start(win_sb[i][:], win_in[i])
                eng.dma_start(s2h_sb[i][:], s2h_in[i])
                eng.dma_start(outp_sb[i][:], outp_in[i])
                eng.dma_start(ddiag_sb[i][:], ddiag_in[i])
                eng.dma_start(cpow_sb[i][:], cpow_in[i].rearrange("a p t -> p a t"))
                eng.dma_start(crev_sb[i][:], crev_in[i].rearrange("a p t -> p a t"))
                if use_ubc:
                    eng.dma_start(ubc_sb[i][:], ubcorr_in[i].rearrange("a p t -> p a t"))

            ubc_sb = []
            for i in range(NB):
                win_sb.append(wts.tile([P, 4 * P], bf16, name=f"win{i}"))
                s2h_sb.append(wts.tile([P, 4 * P], bf16, name=f"s2h{i}"))
                outp_sb.append(wts.tile([P, 4 * P], bf16, name=f"outp{i}"))
                ddiag_sb.append(wts.tile([P, 2 * P], bf16, name=f"ddiag{i}"))
                cpow_sb.append(wts.tile([P, 2, T], bf16, name=f"cpow{i}"))
                crev_sb.append(wts.tile([P, 2, T], bf16, name=f"crev{i}"))
                if use_ubc:
                    ubc_sb.append(wts.tile([P, 2, T], bf16, name=f"ubc{i}"))
            load_layer_weights(0, nc.sync)
            outwt_sb = [wts.tile([P, V], bf16, name=f"outwt{d}") for d in range(2)]
            if use_outb:
                outb_sb = cst.tile([1, V], bf16)
                ones1_sb = cst.tile([1, P], bf16)
                nc.sync.dma_start(ones1_sb[:], onesT[:, :P])

            # ---- embedding gather + pos add ----
            h = wk.tile([P, NT, D], f32)
            for ct in range(NT):
                nc.gpsimd.indirect_dma_start(
                    out=h[:, ct, :], out_offset=None, in_=tok_tab,
                    in_offset=bass.IndirectOffsetOnAxis(ap=ti_sb[:, ct:ct + 1], axis=0))
                nc.vector.tensor_tensor(
                    h[:, ct, :], h[:, ct, :],
                    pos_sb[:, ct * D:(ct + 1) * D], op=OP.add)

            hsT = [None, None]

            # ---- 4 sequential layer passes ----
            for i in range(NB):
                last = i == NB - 1
                # 1. layernorm stats + z = (h - mean) * rstd   (z in bf16)
                z = lay.tile([P, NT, D], bf16, tag="z")
                stats = lay.tile([P, NT, 6], f32, tag="stats")
                aggr = lay.tile([P, NT, 2], f32, tag="aggr")
                rstd = lay.tile([P, NT], f32, tag="rstd")
                negmr = lay.tile([P, NT], f32, tag="negmr")
                # rstd/z run in two ct-halves: the first half's z writes
                # overlap the second half's bn_stats, and each sqrt hop hides
                # behind the other half's vector work
                for hh in range(2):
                    cs = slice(hh * 2, hh * 2 + 2)
                    for ct in range(hh * 2, hh * 2 + 2):
                        nc.vector.bn_stats(stats[:, ct, :], h[:, ct, :])
                        nc.vector.bn_aggr(aggr[:, ct, :], stats[:, ct, :])
                    nc.scalar.activation(rstd[:, cs], aggr[:, cs, 1], AF.Sqrt,
                                         bias=eps_sb[:, 0:1])
                    nc.vector.reciprocal(rstd[:, cs], rstd[:, cs])
                    # negmr = -mean * rstd  (z = h*rstd + negmr)
                    nc.vector.scalar_tensor_tensor(
                        negmr[:, cs], aggr[:, cs, 0], -1.0, rstd[:, cs],
                        op0=OP.mult, op1=OP.mult)
                    for ct in range(hh * 2, hh * 2 + 2):
                        if ct % 2 == 0:
                            nc.scalar.activation(z[:, ct, :], h[:, ct, :],
                                                 AF.Identity,
                                                 bias=negmr[:, ct:ct + 1],
                                                 scale=rstd[:, ct:ct + 1])
                        else:
                            nc.vector.tensor_scalar(z[:, ct, :], h[:, ct, :],
                                                    rstd[:, ct:ct + 1],
                                                    negmr[:, ct:ct + 1],
                                                    op0=OP.mult, op1=OP.add)
                # 2. transpose z -> zT [D part, T free]; ct-outer issue order
                # so the PE FIFO never parks ready transposes behind a z tile
                # that is still being written
                zT = [lay.tile([P, T], bf16, tag=f"zT{d}", name=f"zT{d}") for d in range(2)]
                zt_ps = [ps.tile([P, T], bf16, tag="pp", bufs=8, name=f"zt_ps{d}")
                         for d in range(2)]
                for ct in range(NT):
                    for d in range(2):
                        nc.tensor.transpose(zt_ps[d][:, ct * P:(ct + 1) * P],
                                            z[:, ct, d * P:(d + 1) * P], id_bf[:])
                nc.scalar.activation(zT[0][:], zt_ps[0][:], AF.Identity)
                nc.vector.tensor_copy(zT[1][:], zt_ps[1][:])
                # 3. u = W_in' @ z (+ubias via ones-row matmul); the chunk
                # summary (last scan state) = sum_k c^(T-1-k) u_k computed by a
                # fused multiply+accumulate so the export never waits on a scan
                ns1 = [lay.tile([P, T], bf16, tag=f"ns1{st}", name=f"ns1{st}")
                       for st in range(2)]
                last2 = lay.tile([P, 2], f32, tag="last2")
                scrT = lay.tile([P, T], bf16, tag="scrT")
                u_ps = [None, None]
                for st in range(2):
                    u_ps[st] = ps.tile([P, T], f32, tag="pp", bufs=8, name=f"u_ps{st}")
                    for kt in range(2):
                        nc.tensor.matmul(u_ps[st][:],
                                         win_sb[i][:, (kt * 2 + st) * P:(kt * 2 + st + 1) * P],
                                         zT[kt][:], start=(kt == 0), stop=(kt == 1))
                for st in range(2):
                    nc.vector.scalar_tensor_tensor(
                        scrT[:], u_ps[st][:], 1.0, crev_sb[i][:, st, :],
                        op0=OP.bypass, op1=OP.mult,
                        accum_out=last2[:, st:st + 1])
                # 4. export chunk-final states (transposed: 2 descriptors),
                #    AllGather, combine into carry
                lt_ps = ps.tile([2, P], f32, tag="pp", bufs=8, name="lt_ps")
                nc.tensor.transpose(lt_ps[:], last2[:], id_f[:])
                exp_sb = lay.tile([2, P], bf16, tag="exp")
                nc.scalar.activation(exp_sb[:], lt_ps[:], AF.Identity)
                ag_in = dram.tile([2, P], bf16, name=f"ag_in{i}")
                ag_out = dram.tile([NC, 2, P], bf16, name=f"ag_out{i}",
                                   addr_space="Shared")
                # gpsimd copies the 1KB export itself: the collective trigger
                # is next on the same queue, skipping a cross-engine sem hop
                nc.gpsimd.dma_start(ag_in[:], exp_sb[:])
                if i == 0:
                    # bulk weight loads ride out the collective dead window;
                    # the scalar engine has nothing to do until the carry lands
                    for j in range(1, NB):
                        load_layer_weights(j, nc.scalar)
                    for d in range(2):
                        nc.scalar.dma_start(outwt_sb[d][:], outwt_in[d])
                    if use_outb:
                        nc.scalar.dma_start(outb_sb[:], outb_in)
                # gelu table prefetch sits in the collective dead window; the
                # dep on exp_sb pins it there (a dep on z gets hoisted into
                # the z -> zT -> u critical chain by the scheduler)
                nc.scalar.activation(scrap[:], exp_sb[0:1, 0:1], AF.Gelu_apprx_tanh)
                # full local scans: only the mixed matmuls need them, so they
                # run during the collective wait. The zero initial value is
                # computed from last2 purely to gate the scans behind both
                # summary accumulations — a greedy scheduler otherwise slots a
                # 1.2us scan between them and delays the export.
                z0 = lay.tile([P, 1], f32, tag="z0")
                nc.vector.scalar_tensor_tensor(z0[:], last2[:, 0:1], 0.0,
                                               last2[:, 1:2],
                                               op0=OP.mult, op1=OP.mult)
                for st in range(2):
                    cb = coef_sb[:, i * 2 + st:i * 2 + st + 1].to_broadcast((P, T))
                    nc.vector.tensor_tensor_scan(ns1[st][:], cb, u_ps[st][:],
                                                 z0[:, 0:1],
                                                 op0=OP.mult, op1=OP.add)
                # PE keep-warm: fine-grained dummy matmuls hold the HAM clock
                # gate at 8/8 through the two PE stalls (scan-gate wait, then
                # the collective wait) with <=220ns of FIFO drain each
                # full-bank tile: a fractional PSUM tile can land in a bank
                # with an open m_ps accumulation group and corrupt it
                warm_ps = ps.tile([P, T], f32, tag="pp", bufs=8, name="warm_ps")
                for w in range(N_WARM1):
                    nc.tensor.matmul(warm_ps[:, :P], id_bf[:], zT[0][:, :P],
                                     start=True, stop=True)
                # mixed-psum partials that don't need the carry: issued now so
                # they run during the collective (PE queue is FIFO)
                mixed = [lay.tile([P, T], bf16, tag=f"mix{d}", name=f"mix{d}")
                         for d in range(2)]
                m_ps = [None, None]
                for d in range(2):
                    m_ps[d] = ps.tile([P, T], f32, tag="pp", bufs=8, name=f"m_ps{d}")
                    for st in range(2):
                        nc.tensor.matmul(m_ps[d][:],
                                         s2h_sb[i][:, (st * 2 + d) * P:(st * 2 + d + 1) * P],
                                         ns1[st][:], start=(st == 0), stop=False)
                    nc.tensor.matmul(m_ps[d][:],
                                     ddiag_sb[i][:, d * P:(d + 1) * P],
                                     zT[d][:], start=False, stop=False)
                    if use_ubc:
                        # constant ubias prefix: carry-independent, so it
                        # rides the collective dead window
                        for st in range(2):
                            nc.tensor.matmul(
                                m_ps[d][:],
                                s2h_sb[i][:, (st * 2 + d) * P:(st * 2 + d + 1) * P],
                                ubc_sb[i][:, st, :], start=False, stop=False)
                if last:
                    # h^T transposes are carry-independent: run them in the
                    # collective dead window, not behind the carry matmuls
                    hT_ps_l = []
                    for d2 in range(2):
                        hT = ps.tile([P, T], f32, tag="pp", bufs=8,
                                     name=f"hT_ps{d2}")
                        for ct in range(NT):
                            nc.tensor.transpose(hT[:, ct * P:(ct + 1) * P],
                                                h[:, ct, d2 * P:(d2 + 1) * P],
                                                id_f[:])
                        hT_ps_l.append(hT)
                for w in range(150 if i == 0 else N_WARM2):
                    nc.tensor.matmul(warm_ps[:, :P], id_bf[:], zT[0][:, :P],
                                     start=True, stop=True)
                nc.gpsimd.collective_compute(
                    "AllGather", OP.bypass, replica_groups=[list(range(NC))],
                    ins=[ag_in[:]], outs=[ag_out[:]])
                # gpsimd also copies the gathered summaries back: the copy
                # follows the collective on the same queue, no sem hop
                gath = lay.tile([8, S], bf16, tag="gath")
                nc.gpsimd.dma_start(gath[:], ag_out[:].rearrange("c a b -> c (a b)"))
                q = lay.tile([8, S], f32, tag="q")
                nc.vector.tensor_tensor(q[:], wm_sb[:, i * S:(i + 1) * S], gath[:],
                                        op=OP.mult)
                c_ps = [None, None]
                s2hc = lay.tile([P, 4 * P], bf16, tag="s2hc", name="s2hc")
                for st in range(2):
                    c_ps[st] = ps.tile([P, 1], f32, tag="pp", bufs=8, name=f"c_ps{st}")
                    nc.tensor.matmul(c_ps[st][:], q[:, st * P:(st + 1) * P],
                                     ones8_sb[:], start=True, stop=True)
                    # fold the carry into the s2h weights (s2h diag(carry) @
                    # c^(t+1) == s2h @ (c^(t+1) (.) carry)): one cheap [P,2P]
                    # tensor_scalar instead of building full [P,T] A tables
                    nc.vector.tensor_scalar_mul(
                        s2hc[:, st * 2 * P:(st + 1) * 2 * P],
                        s2h_sb[i][:, st * 2 * P:(st + 1) * 2 * P],
                        c_ps[st][:, 0:1])
                # 6. finish mixed: carry matmuls against the c^(t+1) table,
                # then gelu straight off PSUM
                for d in range(2):
                    for st in range(2):
                        nc.tensor.matmul(m_ps[d][:],
                                         s2hc[:, (st * 2 + d) * P:(st * 2 + d + 1) * P],
                                         cpow_sb[i][:, st, :], start=False, stop=(st == 1))
                    nc.scalar.activation(mixed[d][:], m_ps[d][:], AF.Gelu_apprx_tanh,
                                         bias=db_sb[:, i * 2 + d:i * 2 + d + 1])
                # 6. delta = outp_W' @ mixed (+outp_b); last layer's delta is
                # consumed by a plain f32 add, earlier layers by a PE transpose
                delta = [lay.tile([P, T], f32 if last else bf16,
                                  tag=f"del{d}", name=f"del{d}")
                         for d in range(2)]
                for d2 in range(2):
                    d_ps = ps.tile([P, T], f32, tag="pp", bufs=8, name=f"d_ps{d2}")
                    for d in range(2):
                        nc.tensor.matmul(d_ps[:],
                                         outp_sb[i][:, (d * 2 + d2) * P:(d * 2 + d2 + 1) * P],
                                         mixed[d][:], start=(d == 0), stop=(d == 1))
                    for hh in range(2):
                        sl = slice(hh * (T // 2), (hh + 1) * (T // 2))
                        if d2 == 0:
                            nc.scalar.activation(delta[d2][:, sl], d_ps[:, sl],
                                                 AF.Identity,
                                                 bias=ob_sb[:, i * 2 + d2:i * 2 + d2 + 1])
                        else:
                            nc.vector.tensor_scalar(delta[d2][:, sl], d_ps[:, sl],
                                                    ob_sb[:, i * 2 + d2:i * 2 + d2 + 1],
                                                    None, op0=OP.add)
                if not last:
                    nc.scalar.activation(scrap[:], delta[1][0:1, 0:1], AF.Sqrt)
                # 7. residual
                if not last:
                    for ct in range(NT):
                        dT_ps = ps.tile([P, D], bf16, tag="pp", bufs=8, name="dT_ps")
                        for d2 in range(2):
                            nc.tensor.transpose(dT_ps[:, d2 * P:(d2 + 1) * P],
                                                delta[d2][:, ct * P:(ct + 1) * P], id_bf[:])
                        (nc.vector if ct % 2 == 0 else nc.vector).tensor_tensor(
                            h[:, ct, :], h[:, ct, :], dT_ps[:], op=OP.add)
                else:
                    # hsT = h^T + delta in [D part, T free], bf16 for projection
                    for d2 in range(2):
                        hsT[d2] = wk.tile([P, T], bf16, name=f"hsT{d2}")
                        # per-mt adds: the projection's first lhsT block only
                        # waits for its own 128 columns
                        for ct in range(NT):
                            nc.vector.tensor_tensor(
                                hsT[d2][:, ct * P:(ct + 1) * P],
                                delta[d2][:, ct * P:(ct + 1) * P],
                                hT_ps_l[d2][:, ct * P:(ct + 1) * P], op=OP.add)

            # ---- output projection: out[t, v] = hsT[:, t] . outwt[:, v] ----
            for mt in range(NT):
                for vg in range(NVC // 4):
                    st_t = stg.tile([P, 4 * VC], f16, tag="stg", bufs=4)
                    for vs in range(4):
                        vc = vg * 4 + vs
                        p_ps = ps.tile([P, VC], f32, tag="pp", bufs=8, name="p_ps")
                        for d in range(2):
                            nc.tensor.matmul(p_ps[:], hsT[d][:, mt * P:(mt + 1) * P],
                                             outwt_sb[d][:, vc * VC:(vc + 1) * VC],
                                             start=(d == 0),
                                             stop=(d == 1 and not use_outb))
                        if use_outb:
                            nc.tensor.matmul(p_ps[:], ones1_sb[:],
                                             outb_sb[:, vc * VC:(vc + 1) * VC],
                                             start=False, stop=True)
                        if vc % 2 == 0:
                            nc.vector.tensor_copy(st_t[:, vs * VC:(vs + 1) * VC], p_ps[:])
                        else:
                            nc.scalar.activation(st_t[:, vs * VC:(vs + 1) * VC], p_ps[:],
                                                 AF.Identity)
                    if mt == NT - 1 and vg == NVC // 4 - 1:
                        # final group: two half stores on both rings in
                        # parallel to shorten the drain tail
                        nc.sync.dma_start(
                            out_d[mt * P:(mt + 1) * P,
                                  vg * 4 * VC:vg * 4 * VC + 2 * VC],
                            st_t[:, :2 * VC])
                        nc.scalar.dma_start(
                            out_d[mt * P:(mt + 1) * P,
                                  vg * 4 * VC + 2 * VC:(vg + 1) * 4 * VC],
                            st_t[:, 2 * VC:])
                    else:
                        (nc.sync if vg % 2 == 0 else nc.scalar).dma_start(
                            out_d[mt * P:(mt + 1) * P,
                                  vg * 4 * VC:(vg + 1) * 4 * VC],
                            st_t[:])

    nc.compile()
    _cache[(use_outb, use_ubc)] = nc
    return nc


def _pack_lhsT(w):
    """w: [M, K] weight for out = w @ x. Returns [128, (K/128)*(M/128)*128] lhsT pack;
    block b = kt*nmt + mt holds lhsT[kt*128+p, mt*128+m]."""
    M, K = w.shape
    lhsT = np.ascontiguousarray(w.T)                       # [K, M]
    t = lhsT.reshape(K // P, P, M // P, P)                 # [kt, p, mt, m]
    return np.ascontiguousarray(t.transpose(1, 0, 2, 3).reshape(P, -1))


def kernel(**inputs):
    xs = {k: np.asarray(v) for k, v in inputs.items()}
    tokens = xs["tokens"].astype(np.int32)
    token_embed = xs["token_embed"].astype(np.float32)
    pos_embed = xs["pos_embed"].astype(np.float32)
    in_to_state = xs["in_to_state"].astype(np.float64)
    state_to_hidden = xs["state_to_hidden"].astype(np.float64)
    direct = xs["direct"].astype(np.float64)
    a_diag = xs["a_diag"].astype(np.float64)
    g_diag = xs["g_diag"].astype(np.float64)
    dtp = xs["dt"].astype(np.float64)
    ln_w = xs["ln_w"].astype(np.float64)
    ln_b = xs["ln_b"].astype(np.float64)
    outp_W = xs["outp_W"].astype(np.float64)
    outp_b = xs["outp_b"].astype(np.float32)
    out_W = xs["out_W"].astype(np.float32)
    out_b = xs["out_b"].astype(np.float32)

    def softplus(x):
        return np.logaddexp(0.0, x)

    dt_e = softplus(dtp) + 1e-4
    coeff = np.exp(-softplus(g_diag) * dt_e) * np.cos(a_diag * dt_e)   # [NB, S]
    cdecay = coeff ** T                                                 # [NB, S]
    # c^(t+1) tables for the carry correction, [NB, 2, P, T]
    tpow = np.arange(1, T + 1, dtype=np.float64)
    cpow = coeff.reshape(NB, 2, P, 1) ** tpow.reshape(1, 1, 1, T)
    trev = np.arange(T - 1, -1, -1, dtype=np.float64)
    crev = coeff.reshape(NB, 2, P, 1) ** trev.reshape(1, 1, 1, T)

    import ml_dtypes
    bfl = ml_dtypes.bfloat16
    # packed weights (shared across cores)
    win_pack = np.stack([_pack_lhsT(in_to_state[i] * ln_w[i][None, :]) for i in range(NB)]).astype(bfl)
    s2h_pack = np.stack([_pack_lhsT(state_to_hidden[i]) for i in range(NB)]).astype(bfl)
    outp_pack = np.stack([_pack_lhsT(outp_W[i]) for i in range(NB)]).astype(bfl)
    outwt_pack = np.ascontiguousarray(out_W.T.reshape(2, P, V))
    outwt_bf16 = outwt_pack.astype(bfl)
    ubias = np.stack([in_to_state[i] @ ln_b[i] for i in range(NB)])     # [NB, S]
    # geometric prefixes of coeff: p_t = c p_(t-1) + 1, so the constant ubias
    # feeds the scan as ns_true = scan(u_raw) + ubias * p  (folded into the
    # A-table add) and the chunk summary as + ubias * p_(T-1)
    pref = np.ones((NB, S, T), np.float64)
    for t in range(1, T):
        pref[:, :, t] = coeff * pref[:, :, t - 1] + 1.0
    ubpre = ubias[:, :, None] * pref               # [NB, S, T] local ub prefix
    lbcflat = ubias * pref[:, :, T - 1]            # ub part of every summary
    dprime = direct * ln_w                                              # [NB, D]
    dbias = direct * ln_b                                               # [NB, D]

    def cols(v):  # [NB, 256] -> [128, NB*2] with col (i*2+half)
        return np.ascontiguousarray(
            v.reshape(NB, 2, P).transpose(2, 0, 1).reshape(P, NB * 2)).astype(np.float32)

    use_outb = bool(np.any(out_b != 0.0))

    base = dict(
        tok_tab=token_embed, ident=np.eye(P, dtype=np.float32),
        ident_bf=np.eye(P, dtype=np.float32).astype(bfl),
        ones8=np.ones((8, 1), np.float32),
        onesT=np.ones((1, T), bfl),
        coef_in=cols(coeff),
        ddiag_in=np.ascontiguousarray(np.concatenate(
            [np.stack([np.diag(dprime[i, d * P:(d + 1) * P]) for d in range(2)],
                      axis=1).reshape(P, 2 * P)[None] for i in range(NB)])
        ).astype(bfl),
        dbias_in=cols(dbias),
        opb_in=cols(np.broadcast_to(outp_b, (NB, D)).astype(np.float64)),
        win_in=win_pack, s2h_in=s2h_pack, outp_in=outp_pack,
        cpow_in=cpow.astype(bfl),
        crev_in=crev.astype(bfl),
        outwt_in=outwt_bf16, outb_in=out_b.reshape(1, V).astype(bfl),
    )

    in_maps = []
    for k in range(NC):
        sl = slice(k * T, (k + 1) * T)
        tk = tokens[sl].reshape(NT, P).T.copy()            # [128, NT]
        pos = np.ascontiguousarray(
            pos_embed[sl].reshape(NT, P, D).transpose(1, 0, 2).reshape(P, NT * D))
        # carry weights: wmat[j, s] = cdecay[s]^(k-1-j) for j<k else 0
        wm = np.zeros((8, NB, S), np.float64)
        for j in range(k):
            wm[j] = cdecay ** (k - 1 - j)
        # summaries travel without their (identical) ub part; receivers fold
        # lbc * sum_j(wm) into the carry correction table instead
        corr = lbcflat * wm.sum(0)                 # [NB, S]
        m = dict(base, tok_idx=tk, pos_pre=pos,
                 wmat_in=wm.reshape(8, NB * S).astype(np.float32))
        if np.any(ubias != 0.0):
            ubc_k = ubpre + corr[:, :, None] * cpow.reshape(NB, S, T)
            m["ubcorr_in"] = ubc_k.reshape(NB, 2, P, T).astype(bfl)
        in_maps.append(m)

    use_ubc = bool(np.any(ubias != 0.0))
    nc = _build(use_outb, use_ubc)
    trace = bool(os.environ.get("BASS_KERNEL_TRACE"))
    res = run_bass_kernel_spmd(nc, in_maps, core_ids=list(range(NC)), trace=trace)
    if trace:
        kernel.last_exec_time_ns = res.exec_time_ns
        kernel.last_results = res
    return np.concatenate(
        [res.results[k]["out"].astype(np.float32) for k in range(NC)], axis=0)


# revision 68
# speedup vs baseline: 1.3803x; 1.0524x over previous
"""LocalLinOSS Trainium2 kernel — 8-core SPMD, sequence-sharded.

Model structure (reference): embedding lookup -> 4 sequential blocks; within a
timestep, block i reads the running hidden h (reset to x_t each step), so the
only cross-time recurrence is the per-block diagonal state
    ns_t = coeff (.) ns_{t-1} + in_to_state @ LN(h_t).
The model therefore decomposes into 4 sequential layer passes over the whole
sequence, each = big matmuls over L (parallel) + a first-order linear scan
(hardware tensor_tensor_scan), followed by the [L,D]@[D,V] output projection.

Sharding: L=4096 split into 8 chunks of T=512 (one per core). Per layer, each
core computes a local scan with zero initial state, AllGathers the 8 chunk
final states (1KB payload), combines them into its carry-in using
host-precomputed decay powers (cdecay^j), and applies the carry as a
correction through the next matmul: s2h @ ns = s2h @ ns_local + (s2h
diag(carry)) @ cpow where cpow[s,t] = c_s^(t+1) — the carry folds into the
s2h weights with one cheap tensor_scalar. The two s2h @ ns_local matmuls are
issued before the collective so they overlap it. The output projection runs per-core on its
own T-chunk in bf16 (fp32 accumulate) and stores fp16 (upcast on host).

Layouts: h lives as [T=4x128 part, D free] (layernorm along free dim);
matmul operands live transposed [D or S part, T free]; PE transposes convert.
bf16 feeds all backbone matmuls (fp32 PSUM accumulate); h, the LN stats and
the scan/carry export path stay float32 (the hw scan keeps fp32 state
internally and only downcasts on write). DMA ring budget: sync ring carries
small consts + layer-0 weights + the carry gathers; layers 1-3 and the
projection weights load on the scalar ring during the first collective's
dead window; the 1KB carry export is copied by the gpsimd engine itself so
the collective trigger follows it on the same queue with no cross-engine
sem hop; the gathered summaries return the same way. The constant ubias
contribution to the scan rides host-precomputed ubcorr tables as extra
carry-independent matmuls (skipped entirely when ubias == 0), and chunk
summaries travel without their (identical) ub part. Run-to-run
variance comes almost entirely from cross-core launch skew absorbed at the
first collective; post-collective time is ~135us.
"""
import os
import sys
sys.path.insert(0, "/opt/trn_rl_repo")
import numpy as np
import concourse.bass as bass
import concourse.bacc as bacc
import concourse.mybir as mybir
import concourse.tile as tile
from concourse.bass_utils import run_bass_kernel_spmd

L, D, S, NB, V = 4096, 256, 256, 4, 8000
NC = 8
T = L // NC            # 512 timesteps per core
P = 128
NT = T // P            # 4 T-tiles per core
NVC = 16               # projection V chunks
VC = V // NVC          # 500
f32 = mybir.dt.float32
f32r = mybir.dt.float32r
bf16 = mybir.dt.bfloat16
f16 = mybir.dt.float16
i32 = mybir.dt.int32
AF = mybir.ActivationFunctionType
OP = mybir.AluOpType

N_WARM1 = 16           # PE keep-warm matmuls during the scan-gate wait
N_WARM2 = 40           # PE keep-warm matmuls during the collective wait

_cache = {}


def _build(use_outb: bool, use_ubc: bool):
    if (nc_cached := _cache.get((use_outb, use_ubc))) is not None:
        return nc_cached
    nc = bacc.Bacc("TRN2", target_bir_lowering=False, debug=False,
                   enable_asserts=False, num_devices=NC)

    def din(name, shape, dtype=f32):
        return nc.dram_tensor(name, shape, dtype, kind="ExternalInput").ap()

    tok_idx = din("tok_idx", [P, NT], i32)
    tok_tab = din("tok_tab", [V, D])
    pos_pre = din("pos_pre", [P, NT * D])
    ident = din("ident", [P, P])
    ident_bf = din("ident_bf", [P, P], bf16)
    ones8 = din("ones8", [8, 1])
    onesT = din("onesT", [1, T], bf16)
    coef_in = din("coef_in", [P, NB * 2])          # coeff, col (i*2+st)
    wmat_in = din("wmat_in", [8, NB * S])          # per-core carry weights
    ubcorr_in = (din("ubcorr_in", [NB, 2, P, T], bf16)  # ub/carry consts
                 if use_ubc else None)
    ddiag_in = din("ddiag_in", [NB, P, 2 * P], bf16)   # diag(direct * ln_w) lhsT
    dbias_in = din("dbias_in", [P, NB * 2])        # direct * ln_b
    opb_in = din("opb_in", [P, NB * 2])            # outp_b
    win_in = din("win_in", [NB, P, 4 * P], bf16)   # lhsT packs
    s2h_in = din("s2h_in", [NB, P, 4 * P], bf16)
    outp_in = din("outp_in", [NB, P, 4 * P], bf16)
    cpow_in = din("cpow_in", [NB, 2, P, T], bf16)  # c^(t+1)
    crev_in = din("crev_in", [NB, 2, P, T], bf16)  # c^(T-1-t)
    outwt_in = din("outwt_in", [2, P, V], bf16)
    outb_in = din("outb_in", [1, V], bf16)
    out_d = nc.dram_tensor("out", [T, V], f16, kind="ExternalOutput").ap()

    with tile.TileContext(nc) as tc:
        with tc.tile_pool(name="const", bufs=1) as cst, \
             tc.tile_pool(name="wts", bufs=1) as wts, \
             tc.tile_pool(name="work", bufs=1) as wk, \
             tc.tile_pool(name="lay", bufs=2) as lay, \
             tc.tile_pool(name="psum", bufs=1, space="PSUM") as ps, \
             tc.tile_pool(name="stage", bufs=1) as stg, \
             tc.tile_pool(name="dram", bufs=1, space="DRAM") as dram:

            # ---- loads, ordered by when they are first needed ----
            ti_sb = wk.tile([P, NT], i32)
            nc.sync.dma_start(ti_sb[:], tok_idx)
            id_f = cst.tile([P, P], f32)
            nc.sync.dma_start(id_f[:], ident)
            id_bf = cst.tile([P, P], bf16)
            nc.sync.dma_start(id_bf[:], ident_bf)
            pos_sb = wk.tile([P, NT * D], f32)
            nc.sync.dma_start(pos_sb[:], pos_pre)
            coef_sb = cst.tile([P, NB * 2], f32)
            nc.sync.dma_start(coef_sb[:], coef_in)
            ones8_sb = cst.tile([8, 1], f32)
            nc.sync.dma_start(ones8_sb[:], ones8)
            wm_sb = cst.tile([8, NB * S], f32)
            nc.sync.dma_start(wm_sb[:], wmat_in)
            db_sb = cst.tile([P, NB * 2], f32)
            nc.sync.dma_start(db_sb[:], dbias_in)
            ob_sb = cst.tile([P, NB * 2], f32)
            nc.sync.dma_start(ob_sb[:], opb_in)
            scrap = cst.tile([1, 1], f32)          # dummy act target
            # prefetch the Identity and Sqrt activation tables while the
            # scalar engine is idle, so layer 0's z/rstd chain never waits;
            # touch the vector queue too so its bring-up cost is paid early
            nc.scalar.activation(scrap[:], id_f[0:1, 0:1], AF.Identity)
            nc.scalar.activation(scrap[:], id_f[0:1, 0:1], AF.Sqrt)
            scrapv = cst.tile([1, 1], f32)
            nc.vector.tensor_copy(scrapv[:], id_f[0:1, 0:1])
            eps_sb = cst.tile([P, 1], f32)
            nc.vector.memset(eps_sb[:], 1e-5)
            # layer-0 weights ride the sync ring now (behind the small
            # consts); layers 1-3 + projection weights are deferred into the
            # first collective's dead window (triggers issued further down on
            # the scalar ring, which sits idle then anyway)
            win_sb, s2h_sb, outp_sb, cpow_sb, ddiag_sb, crev_sb = [], [], [], [], [], []

            def load_layer_weights(i, eng):
                eng.dma_start(win_sb[i][:], win_in[i])
                eng.dma_start(s2h_sb[i][:], s2h_in[i])
                eng.dma_start(outp_sb[i][:], outp_in[i])
                eng.dma_start(ddiag_sb[i][:], ddiag_in[i])
                eng.dma_start(cpow_sb[i][:], cpow_in[i].rearrange("a p t -> p a t"))
                eng.dma_start(crev_sb[i][:], crev_in[i].rearrange("a p t -> p a t"))
                if use_ubc:
                    eng.dma_start(ubc_sb[i][:], ubcorr_in[i].rearrange("a p t -> p a t"))

            ubc_sb = []
            for i in range(NB):
                win_sb.append(wts.tile([P, 4 * P], bf16, name=f"win{i}"))
                s2h_sb.append(wts.tile([P, 4 * P], bf16, name=f"s2h{i}"))
                outp_sb.append(wts.tile([P, 4 * P], bf16, name=f"outp{i}"))
                ddiag_sb.append(wts.tile([P, 2 * P], bf16, name=f"ddiag{i}"))
                cpow_sb.append(wts.tile([P, 2, T], bf16, name=f"cpow{i}"))
                crev_sb.append(wts.tile([P, 2, T], bf16, name=f"crev{i}"))
                if use_ubc:
                    ubc_sb.append(wts.tile([P, 2, T], bf16, name=f"ubc{i}"))
            load_layer_weights(0, nc.sync)
            outwt_sb = [wts.tile([P, V], bf16, name=f"outwt{d}") for d in range(2)]
            if use_outb:
                outb_sb = cst.tile([1, V], bf16)
                ones1_sb = cst.tile([1, P], bf16)
                nc.sync.dma_start(ones1_sb[:], onesT[:, :P])

            # ---- embedding gather + pos add ----
            h = wk.tile([P, NT, D], f32)
            for ct in range(NT):
                nc.gpsimd.indirect_dma_start(
                    out=h[:, ct, :], out_offset=None, in_=tok_tab,
                    in_offset=bass.IndirectOffsetOnAxis(ap=ti_sb[:, ct:ct + 1], axis=0))
                nc.vector.tensor_tensor(
                    h[:, ct, :], h[:, ct, :],
                    pos_sb[:, ct * D:(ct + 1) * D], op=OP.add)

            hsT = [None, None]

            # ---- 4 sequential layer passes ----
            for i in range(NB):
                last = i == NB - 1
                # 1. layernorm stats + z = (h - mean) * rstd   (z in bf16)
                z = lay.tile([P, NT, D], bf16, tag="z")
                stats = lay.tile([P, NT, 6], f32, tag="stats")
                aggr = lay.tile([P, NT, 2], f32, tag="aggr")
                rstd = lay.tile([P, NT], f32, tag="rstd")
                negmr = lay.tile([P, NT], f32, tag="negmr")
                # rstd/z run in two ct-halves: the first half's z writes
                # overlap the second half's bn_stats, and each sqrt hop hides
                # behind the other half's vector work
                for hh in range(2):
                    cs = slice(hh * 2, hh * 2 + 2)
                    for ct in range(hh * 2, hh * 2 + 2):
                        nc.vector.bn_stats(stats[:, ct, :], h[:, ct, :])
                        nc.vector.bn_aggr(aggr[:, ct, :], stats[:, ct, :])
                    nc.scalar.activation(rstd[:, cs], aggr[:, cs, 1], AF.Sqrt,
                                         bias=eps_sb[:, 0:1])
                    nc.vector.reciprocal(rstd[:, cs], rstd[:, cs])
                    # negmr = -mean * rstd  (z = h*rstd + negmr)
                    nc.vector.scalar_tensor_tensor(
                        negmr[:, cs], aggr[:, cs, 0], -1.0, rstd[:, cs],
                        op0=OP.mult, op1=OP.mult)
                    for ct in range(hh * 2, hh * 2 + 2):
                        if ct % 2 == 0:
                            nc.scalar.activation(z[:, ct, :], h[:, ct, :],
                                                 AF.Identity,
                                                 bias=negmr[:, ct:ct + 1],
                                                 scale=rstd[:, ct:ct + 1])
                        else:
                            nc.vector.tensor_scalar(z[:, ct, :], h[:, ct, :],
                                                    rstd[:, ct:ct + 1],
                                                    negmr[:, ct:ct + 1],
                                                    op0=OP.mult, op1=OP.add)
                # 2. transpose z -> zT [D part, T free]; ct-outer issue order
                # so the PE FIFO never parks ready transposes behind a z tile
                # that is still being written
                zT = [lay.tile([P, T], bf16, tag=f"zT{d}", name=f"zT{d}") for d in range(2)]
                zt_ps = [ps.tile([P, T], bf16, tag="pp", bufs=8, name=f"zt_ps{d}")
                         for d in range(2)]
                for ct in range(NT):
                    for d in range(2):
                        nc.tensor.transpose(zt_ps[d][:, ct * P:(ct + 1) * P],
                                            z[:, ct, d * P:(d + 1) * P], id_bf[:])
                nc.scalar.activation(zT[0][:], zt_ps[0][:], AF.Identity)
                nc.vector.tensor_copy(zT[1][:], zt_ps[1][:])
                # 3. u = W_in' @ z (+ubias via ones-row matmul); the chunk
                # summary (last scan state) = sum_k c^(T-1-k) u_k computed by a
                # fused multiply+accumulate so the export never waits on a scan
                ns1 = [lay.tile([P, T], bf16, tag=f"ns1{st}", name=f"ns1{st}")
                       for st in range(2)]
                last2 = lay.tile([P, 2], f32, tag="last2")
                scrT = lay.tile([P, T], bf16, tag="scrT")
                u_ps = [None, None]
                for st in range(2):
                    u_ps[st] = ps.tile([P, T], f32, tag="pp", bufs=8, name=f"u_ps{st}")
                    for kt in (1, 0):
                        nc.tensor.matmul(u_ps[st][:],
                                         win_sb[i][:, (kt * 2 + st) * P:(kt * 2 + st + 1) * P],
                                         zT[kt][:], start=(kt == 1), stop=(kt == 0))
                for st in range(2):
                    nc.vector.scalar_tensor_tensor(
                        scrT[:], u_ps[st][:], 1.0, crev_sb[i][:, st, :],
                        op0=OP.bypass, op1=OP.mult,
                        accum_out=last2[:, st:st + 1])
                # 4. export chunk-final states (transposed: 2 descriptors),
                #    AllGather, combine into carry
                lt_ps = ps.tile([2, P], f32, tag="pp", bufs=8, name="lt_ps")
                nc.tensor.transpose(lt_ps[:], last2[:], id_f[:])
                exp_sb = lay.tile([2, P], bf16, tag="exp")
                nc.scalar.activation(exp_sb[:], lt_ps[:], AF.Identity)
                ag_in = dram.tile([2, P], bf16, name=f"ag_in{i}")
                ag_out = dram.tile([NC, 2, P], bf16, name=f"ag_out{i}",
                                   addr_space="Shared")
                # gpsimd copies the 1KB export itself: the collective trigger
                # is next on the same queue, skipping a cross-engine sem hop
                nc.gpsimd.dma_start(ag_in[:], exp_sb[:])
                if i == 0:
                    # bulk weight loads ride out the collective dead window;
                    # the scalar engine has nothing to do until the carry lands
                    for j in range(1, NB):
                        load_layer_weights(j, nc.scalar)
                    for d in range(2):
                        nc.scalar.dma_start(outwt_sb[d][:], outwt_in[d])
                    if use_outb:
                        nc.scalar.dma_start(outb_sb[:], outb_in)
                # gelu table prefetch sits in the collective dead window; the
                # dep on exp_sb pins it there (a dep on z gets hoisted into
                # the z -> zT -> u critical chain by the scheduler)
                nc.scalar.activation(scrap[:], exp_sb[0:1, 0:1], AF.Gelu_apprx_tanh)
                # full local scans: only the mixed matmuls need them, so they
                # run during the collective wait. The zero initial value is
                # computed from last2 purely to gate the scans behind both
                # summary accumulations — a greedy scheduler otherwise slots a
                # 1.2us scan between them and delays the export.
                z0 = lay.tile([P, 1], f32, tag="z0")
                nc.vector.scalar_tensor_tensor(z0[:], last2[:, 0:1], 0.0,
                                               last2[:, 1:2],
                                               op0=OP.mult, op1=OP.mult)
                for st in range(2):
                    cb = coef_sb[:, i * 2 + st:i * 2 + st + 1].to_broadcast((P, T))
                    nc.vector.tensor_tensor_scan(ns1[st][:], cb, u_ps[st][:],
                                                 z0[:, 0:1],
                                                 op0=OP.mult, op1=OP.add)
                # PE keep-warm: fine-grained dummy matmuls hold the HAM clock
                # gate at 8/8 through the two PE stalls (scan-gate wait, then
                # the collective wait) with <=220ns of FIFO drain each
                # full-bank tile: a fractional PSUM tile can land in a bank
                # with an open m_ps accumulation group and corrupt it
                warm_ps = ps.tile([P, T], f32, tag="pp", bufs=8, name="warm_ps")
                for w in range(N_WARM1):
                    nc.tensor.matmul(warm_ps[:, :P], id_bf[:], zT[0][:, :P],
                                     start=True, stop=True)
                # mixed-psum partials that don't need the carry: issued now so
                # they run during the collective (PE queue is FIFO)
                mixed = [lay.tile([P, T], bf16, tag=f"mix{d}", name=f"mix{d}")
                         for d in range(2)]
                m_ps = [None, None]
                for d in range(2):
                    m_ps[d] = ps.tile([P, T], f32, tag="pp", bufs=8, name=f"m_ps{d}")
                    for st in range(2):
                        nc.tensor.matmul(m_ps[d][:],
                                         s2h_sb[i][:, (st * 2 + d) * P:(st * 2 + d + 1) * P],
                                         ns1[st][:], start=(st == 0), stop=False)
                    nc.tensor.matmul(m_ps[d][:],
                                     ddiag_sb[i][:, d * P:(d + 1) * P],
                                     zT[d][:], start=False, stop=False)
                    if use_ubc:
                        # constant ubias prefix: carry-independent, so it
                        # rides the collective dead window
                        for st in range(2):
                            nc.tensor.matmul(
                                m_ps[d][:],
                                s2h_sb[i][:, (st * 2 + d) * P:(st * 2 + d + 1) * P],
                                ubc_sb[i][:, st, :], start=False, stop=False)
                if last:
                    # h^T transposes are carry-independent: run them in the
                    # collective dead window, not behind the carry matmuls
                    hT_ps_l = []
                    for d2 in range(2):
                        hT = ps.tile([P, T], f32, tag="pp", bufs=8,
                                     name=f"hT_ps{d2}")
                        for ct in range(NT):
                            nc.tensor.transpose(hT[:, ct * P:(ct + 1) * P],
                                                h[:, ct, d2 * P:(d2 + 1) * P],
                                                id_f[:])
                        hT_ps_l.append(hT)
                for w in range(150 if i == 0 else N_WARM2):
                    nc.tensor.matmul(warm_ps[:, :P], id_bf[:], zT[0][:, :P],
                                     start=True, stop=True)
                nc.gpsimd.collective_compute(
                    "AllGather", OP.bypass, replica_groups=[list(range(NC))],
                    ins=[ag_in[:]], outs=[ag_out[:]])
                # gpsimd also copies the gathered summaries back: the copy
                # follows the collective on the same queue, no sem hop
                gath = lay.tile([8, S], bf16, tag="gath")
                nc.gpsimd.dma_start(gath[:], ag_out[:].rearrange("c a b -> c (a b)"))
                q = lay.tile([8, S], f32, tag="q")
                nc.vector.tensor_tensor(q[:], wm_sb[:, i * S:(i + 1) * S], gath[:],
                                        op=OP.mult)
                c_ps = [None, None]
                s2hc = lay.tile([P, 4 * P], bf16, tag="s2hc", name="s2hc")
                for st in range(2):
                    c_ps[st] = ps.tile([P, 1], f32, tag="pp", bufs=8, name=f"c_ps{st}")
                    nc.tensor.matmul(c_ps[st][:], q[:, st * P:(st + 1) * P],
                                     ones8_sb[:], start=True, stop=True)
                    # fold the carry into the s2h weights (s2h diag(carry) @
                    # c^(t+1) == s2h @ (c^(t+1) (.) carry)): one cheap [P,2P]
                    # tensor_scalar instead of building full [P,T] A tables
                    nc.vector.tensor_scalar_mul(
                        s2hc[:, st * 2 * P:(st + 1) * 2 * P],
                        s2h_sb[i][:, st * 2 * P:(st + 1) * 2 * P],
                        c_ps[st][:, 0:1])
                # 6. finish mixed: carry matmuls against the c^(t+1) table,
                # then gelu straight off PSUM
                for d in range(2):
                    for st in range(2):
                        nc.tensor.matmul(m_ps[d][:],
                                         s2hc[:, (st * 2 + d) * P:(st * 2 + d + 1) * P],
                                         cpow_sb[i][:, st, :], start=False, stop=(st == 1))
                    nc.scalar.activation(mixed[d][:], m_ps[d][:], AF.Gelu_apprx_tanh,
                                         bias=db_sb[:, i * 2 + d:i * 2 + d + 1])
                # 6. delta = outp_W' @ mixed (+outp_b); last layer's delta is
                # consumed by a plain f32 add, earlier layers by a PE transpose
                delta = [lay.tile([P, T], f32 if last else bf16,
                                  tag=f"del{d}", name=f"del{d}")
                         for d in range(2)]
                for d2 in range(2):
                    d_ps = ps.tile([P, T], f32, tag="pp", bufs=8, name=f"d_ps{d2}")
                    for d in range(2):
                        nc.tensor.matmul(d_ps[:],
                                         outp_sb[i][:, (d * 2 + d2) * P:(d * 2 + d2 + 1) * P],
                                         mixed[d][:], start=(d == 0), stop=(d == 1))
                    for hh in range(2):
                        sl = slice(hh * (T // 2), (hh + 1) * (T // 2))
                        if d2 == 0:
                            nc.scalar.activation(delta[d2][:, sl], d_ps[:, sl],
                                                 AF.Identity,
                                                 bias=ob_sb[:, i * 2 + d2:i * 2 + d2 + 1])
                        else:
                            nc.vector.tensor_scalar(delta[d2][:, sl], d_ps[:, sl],
                                                    ob_sb[:, i * 2 + d2:i * 2 + d2 + 1],
                                                    None, op0=OP.add)
                if not last:
                    nc.scalar.activation(scrap[:], delta[1][0:1, 0:1], AF.Sqrt)
                # 7. residual
                if not last:
                    for ct in range(NT):
                        dT_ps = ps.tile([P, D], bf16, tag="pp", bufs=8, name="dT_ps")
                        for d2 in range(2):
                            nc.tensor.transpose(dT_ps[:, d2 * P:(d2 + 1) * P],
                                                delta[d2][:, ct * P:(ct + 1) * P], id_bf[:])
                        (nc.vector if ct % 2 == 0 else nc.vector).tensor_tensor(
                            h[:, ct, :], h[:, ct, :], dT_ps[:], op=OP.add)
                else:
                    # hsT = h^T + delta in [D part, T free], bf16 for the
                    # projection; ct-outer so projection mt0 unblocks after
                    # two adds instead of five
                    for d2 in range(2):
                        hsT[d2] = wk.tile([P, T], bf16, name=f"hsT{d2}")
                    for ct in range(NT):
                        for d2 in range(2):
                            nc.vector.tensor_tensor(
                                hsT[d2][:, ct * P:(ct + 1) * P],
                                delta[d2][:, ct * P:(ct + 1) * P],
                                hT_ps_l[d2][:, ct * P:(ct + 1) * P], op=OP.add)

            # ---- output projection: out[t, v] = hsT[:, t] . outwt[:, v] ----
            for mt in range(NT):
                for vg in range(NVC // 4):
                    st_t = stg.tile([P, 4 * VC], f16, tag="stg", bufs=4)
                    for vs in range(4):
                        vc = vg * 4 + vs
                        p_ps = ps.tile([P, VC], f32, tag="pp", bufs=8, name="p_ps")
                        for d in range(2):
                            nc.tensor.matmul(p_ps[:], hsT[d][:, mt * P:(mt + 1) * P],
                                             outwt_sb[d][:, vc * VC:(vc + 1) * VC],
                                             start=(d == 0),
                                             stop=(d == 1 and not use_outb))
                        if use_outb:
                            nc.tensor.matmul(p_ps[:], ones1_sb[:],
                                             outb_sb[:, vc * VC:(vc + 1) * VC],
                                             start=False, stop=True)
                        if vc % 2 == 0:
                            nc.vector.tensor_copy(st_t[:, vs * VC:(vs + 1) * VC], p_ps[:])
                        else:
                            nc.scalar.activation(st_t[:, vs * VC:(vs + 1) * VC], p_ps[:],
                                                 AF.Identity)
                    if mt == NT - 1 and vg == NVC // 4 - 1:
                        # final group: two half stores on both rings in
                        # parallel to shorten the drain tail
                        nc.sync.dma_start(
                            out_d[mt * P:(mt + 1) * P,
                                  vg * 4 * VC:vg * 4 * VC + 2 * VC],
                            st_t[:, :2 * VC])
                        nc.scalar.dma_start(
                            out_d[mt * P:(mt + 1) * P,
                                  vg * 4 * VC + 2 * VC:(vg + 1) * 4 * VC],
                            st_t[:, 2 * VC:])
                    else:
                        (nc.sync if vg % 2 == 0 else nc.scalar).dma_start(
                            out_d[mt * P:(mt + 1) * P,
                                  vg * 4 * VC:(vg + 1) * 4 * VC],
                            st_t[:])

    nc.compile()
    _cache[(use_outb, use_ubc)] = nc
    return nc


def _pack_lhsT(w):
    """w: [M, K] weight for out = w @ x. Returns [128, (K/128)*(M/128)*128] lhsT pack;
    block b = kt*nmt + mt holds lhsT[kt*128+p, mt*128+m]."""
    M, K = w.shape
    lhsT = np.ascontiguousarray(w.T)                       # [K, M]
    t = lhsT.reshape(K // P, P, M // P, P)                 # [kt, p, mt, m]
    return np.ascontiguousarray(t.transpose(1, 0, 2, 3).reshape(P, -1))


def kernel(**inputs):
    xs = {k: np.asarray(v) for k, v in inputs.items()}
    tokens = xs["tokens"].astype(np.int32)
    token_embed = xs["token_embed"].astype(np.float32)
    pos_embed = xs["pos_embed"].astype(np.float32)
    in_to_state = xs["in_to_state"].astype(np.float64)
    state_to_hidden = xs["state_to_hidden"].astype(np.float64)
    direct = xs["direct"].astype(np.float64)
    a_diag = xs["a_diag"].astype(np.float64)
    g_diag = xs["g_diag"].astype(np.float64)
    dtp = xs["dt"].astype(np.float64)
    ln_w = xs["ln_w"].astype(np.float64)
    ln_b = xs["ln_b"].astype(np.float64)
    outp_W = xs["outp_W"].astype(np.float64)
    outp_b = xs["outp_b"].astype(np.float32)
    out_W = xs["out_W"].astype(np.float32)
    out_b = xs["out_b"].astype(np.float32)

    def softplus(x):
        return np.logaddexp(0.0, x)

    dt_e = softplus(dtp) + 1e-4
    coeff = np.exp(-softplus(g_diag) * dt_e) * np.cos(a_diag * dt_e)   # [NB, S]
    cdecay = coeff ** T                                                 # [NB, S]
    # c^(t+1) tables for the carry correction, [NB, 2, P, T]
    tpow = np.arange(1, T + 1, dtype=np.float64)
    cpow = coeff.reshape(NB, 2, P, 1) ** tpow.reshape(1, 1, 1, T)
    trev = np.arange(T - 1, -1, -1, dtype=np.float64)
    crev = coeff.reshape(NB, 2, P, 1) ** trev.reshape(1, 1, 1, T)

    import ml_dtypes
    bfl = ml_dtypes.bfloat16
    # packed weights (shared across cores)
    win_pack = np.stack([_pack_lhsT(in_to_state[i] * ln_w[i][None, :]) for i in range(NB)]).astype(bfl)
    s2h_pack = np.stack([_pack_lhsT(state_to_hidden[i]) for i in range(NB)]).astype(bfl)
    outp_pack = np.stack([_pack_lhsT(outp_W[i]) for i in range(NB)]).astype(bfl)
    outwt_pack = np.ascontiguousarray(out_W.T.reshape(2, P, V))
    outwt_bf16 = outwt_pack.astype(bfl)
    ubias = np.stack([in_to_state[i] @ ln_b[i] for i in range(NB)])     # [NB, S]
    # geometric prefixes of coeff: p_t = c p_(t-1) + 1, so the constant ubias
    # feeds the scan as ns_true = scan(u_raw) + ubias * p  (folded into the
    # A-table add) and the chunk summary as + ubias * p_(T-1)
    pref = np.ones((NB, S, T), np.float64)
    for t in range(1, T):
        pref[:, :, t] = coeff * pref[:, :, t - 1] + 1.0
    ubpre = ubias[:, :, None] * pref               # [NB, S, T] local ub prefix
    lbcflat = ubias * pref[:, :, T - 1]            # ub part of every summary
    dprime = direct * ln_w                                              # [NB, D]
    dbias = direct * ln_b                                               # [NB, D]

    def cols(v):  # [NB, 256] -> [128, NB*2] with col (i*2+half)
        return np.ascontiguousarray(
            v.reshape(NB, 2, P).transpose(2, 0, 1).reshape(P, NB * 2)).astype(np.float32)

    use_outb = bool(np.any(out_b != 0.0))

    base = dict(
        tok_tab=token_embed, ident=np.eye(P, dtype=np.float32),
        ident_bf=np.eye(P, dtype=np.float32).astype(bfl),
        ones8=np.ones((8, 1), np.float32),
        onesT=np.ones((1, T), bfl),
        coef_in=cols(coeff),
        ddiag_in=np.ascontiguousarray(np.concatenate(
            [np.stack([np.diag(dprime[i, d * P:(d + 1) * P]) for d in range(2)],
                      axis=1).reshape(P, 2 * P)[None] for i in range(NB)])
        ).astype(bfl),
        dbias_in=cols(dbias),
        opb_in=cols(np.broadcast_to(outp_b, (NB, D)).astype(np.float64)),
        win_in=win_pack, s2h_in=s2h_pack, outp_in=outp_pack,
        cpow_in=cpow.astype(bfl),
        crev_in=crev.astype(bfl),
        outwt_in=outwt_bf16, outb_in=out_b.reshape(1, V).astype(bfl),
    )

    in_maps = []
    for k in range(NC):
        sl = slice(k * T, (k + 1) * T)
        tk = tokens[sl].reshape(NT, P).T.copy()            # [128, NT]
        pos = np.ascontiguousarray(
            pos_embed[sl].reshape(NT, P, D).transpose(1, 0, 2).reshape(P, NT * D))
        # carry weights: wmat[j, s] = cdecay[s]^(k-1-j) for j<k else 0
        wm = np.zeros((8, NB, S), np.float64)
        for j in range(k):
            wm[j] = cdecay ** (k - 1 - j)
        # summaries travel without their (identical) ub part; receivers fold
        # lbc * sum_j(wm) into the carry correction table instead
        corr = lbcflat * wm.sum(0)                 # [NB, S]
        m = dict(base, tok_idx=tk, pos_pre=pos,
                 wmat_in=wm.reshape(8, NB * S).astype(np.float32))
        if np.any(ubias != 0.0):
            ubc_k = ubpre + corr[:, :, None] * cpow.reshape(NB, S, T)
            m["ubcorr_in"] = ubc_k.reshape(NB, 2, P, T).astype(bfl)
        in_maps.append(m)

    use_ubc = bool(np.any(ubias != 0.0))
    nc = _build(use_outb, use_ubc)
    trace = bool(os.environ.get("BASS_KERNEL_TRACE"))
    res = run_bass_kernel_spmd(nc, in_maps, core_ids=list(range(NC)), trace=trace)
    if trace:
        kernel.last_exec_time_ns = res.exec_time_ns
        kernel.last_results = res
    return np.concatenate(
        [res.results[k]["out"].astype(np.float32) for k in range(NC)], axis=0)
